# revision 17
# baseline (speedup 1.0000x reference)
"""Decode-step attention-partition kernel for 8 Trainium2 NeuronCores.

Shape (hardcoded from the problem spec):
  x[16,1,4096], ln_w[4096], Wq/Wk/Wv/Wo[4096,4096],
  K_cache/V_cache[16,2048,32,128], cache_lens[16] int32.

Sharding: head-parallel. Core c owns heads [4c, 4c+4) for ALL 16 requests:
  - RMSNorm of x replicated (tiny).
  - q/k computed directly in transposed form qT/kT [128d, 16b] per head;
    v in natural form [16, 512].
  - Ragged attention over the per-request cache with the new token folded in
    at position L_b (kT column spliced into the transposed K tile; v row
    spliced into the V tile). Softmax without max-subtraction (scores are
    O(10) here so exp() is safe in fp32); normalization 1/Z folded into the
    per-head o_proj epilogue as a per-partition scale.
  - Each core emits a partial o_proj [16, 4096] (its 4 heads' contribution);
    the host sums the 8 partials and adds the residual.

Request lengths are read on the host and baked into the instruction stream
(static trip counts, exact-size DMAs). Requests are sorted by length
descending so per-t-block "valid request" sets are prefixes.
"""

import sys
import types
import ctypes
import contextlib

import numpy as np

# ---------------------------------------------------------------------------
# axon NTFF profile hook (the image's antenv lacks axon_hooks; the capability
# exists in libaxon_pjrt.so). Registered before concourse.bass_utils import.
# ---------------------------------------------------------------------------


def _install_ntff_hook():
    if "antenv.axon_hooks" in sys.modules:
        return
    try:
        lib = ctypes.CDLL("/opt/axon/libaxon_pjrt.so")
        lib.axon_start_nrt_profile.argtypes = [
            ctypes.POINTER(ctypes.c_int64),
            ctypes.c_size_t,
        ]
        lib.axon_start_nrt_profile.restype = ctypes.c_int64
        lib.axon_stop_nrt_profile.argtypes = [ctypes.c_char_p]
        lib.axon_stop_nrt_profile.restype = ctypes.c_int64
    except OSError:
        lib = None

    @contextlib.contextmanager
    def _hook(output_dir, device_ids):
        import jax

        jax.devices()
        if device_ids:
            ids = (ctypes.c_int64 * len(device_ids))(*device_ids)
            rc = lib.axon_start_nrt_profile(ids, len(device_ids))
        else:
            rc = lib.axon_start_nrt_profile(None, 0)
        if rc != 0:
            raise RuntimeError(f"axon_start_nrt_profile rc={rc}")
        try:
            yield
        finally:
            n = lib.axon_stop_nrt_profile(str(output_dir).encode())
            print(f"ntff profile: {n} file(s) -> {output_dir}", file=sys.stderr)

    mod = types.ModuleType("antenv.axon_hooks")
    mod.get_axon_ntff_profile_hook = (lambda: _hook) if lib is not None else (lambda: None)
    mod.set_axon_ntff_profile_hook = lambda h: None
    sys.modules["antenv.axon_hooks"] = mod


_install_ntff_hook()

import concourse.bass as bass
import concourse.mybir as mybir
import concourse.tile as tile
from concourse.vector_clock import ScopedClock
from concourse.masks import make_identity
from concourse.bass_utils import run_bass_kernel_spmd

# ---------------------------------------------------------------------------
# This walrus build rejects instructions with >1 semaphore wait command
# ("Too many sync wait commands" in setupSyncWait for CTRL structs). Tile's
# kernel-tail drain accumulates one wait per engine/DMA lane. Split the waits
# across preceding same-engine NOPs (1 wait each).
# ---------------------------------------------------------------------------
_MAXW = 1


def _patched_drain_and_barrier(self, tick_clock, wait_clock):
    nc = self.nc
    probe = nc.sync.nop(nofuse=True)
    wait_clock.add_sem_waits(probe.ins, ScopedClock({None: tick_clock.global_clock}))
    si = probe.ins.sync_info
    waits = list(si.on_wait) if si is not None else []
    if len(waits) > _MAXW:
        si.on_wait = waits[:_MAXW]
        for i in range(_MAXW, len(waits), _MAXW):
            nop = nc.sync.nop(nofuse=True)
            nop.ins.sync_info = mybir.SyncInfo(
                on_wait=waits[i : i + _MAXW], on_update=[]
            )
    nc.sync.drain()
    nc.all_engine_barrier()
    assert self.sems is not None
    popped = nc._tile_sem_poison_stack.pop()
    assert popped is self._sem_poison
    nc.clear_and_free_semaphores(list(self.sems.allocated().values()))
    nc.all_engine_barrier()


tile.TileContext._drain_and_barrier = _patched_drain_and_barrier

_wsplit_counter = [0]


def _split_excess_waits(nc):
    """Post-pass: this walrus build allows at most 1 sem-wait per instruction.
    Move excess waits onto preceding same-engine NoOps (same-engine program
    order preserves the wait semantics)."""
    for fn in nc.m.functions:
        for bb in fn.blocks:
            out = []
            changed = False
            for inst in bb.instructions:
                si = inst.sync_info
                if (
                    si is not None
                    and len(si.on_wait) > 1
                    and not isinstance(inst, mybir.InstAllEngineBarrier)
                ):
                    waits = list(si.on_wait)
                    for w in waits[:-1]:
                        _wsplit_counter[0] += 1
                        out.append(
                            mybir.InstNoOp(
                                name=f"I-wsplit-{_wsplit_counter[0]}",
                                engine=inst.engine,
                                sync_info=mybir.SyncInfo(
                                    on_wait=[w], on_update=[]
                                ),
                            )
                        )
                    si.on_wait = [waits[-1]]
                    changed = True
                out.append(inst)
            if changed:
                bb.instructions[:] = out

# ---------------------------------------------------------------------------

F32 = mybir.dt.float32
BF16 = mybir.dt.bfloat16
P = 128
B = 16
T = 2048
D = 4096
H = 32
HD = 128
NHL = 4          # heads per core
NCORES = 8
EPS = 1e-6
NKC = D // P     # 32 contraction chunks for the projections
SCALE = 1.0 / float(np.sqrt(HD))
JG = 4           # t-blocks per K/V DMA group (4*128 rows x 2KB = 1 MiB)


def _build(Ls):
    """Build the per-core Bass kernel. Ls: 16 request lengths, sorted desc."""
    nblk = [l // P + 1 for l in Ls]          # t-blocks incl. the new token
    r = [l % P for l in Ls]                  # new-token row within tail block
    vt = [rr + 1 for rr in r]                # valid rows in tail block
    jmax = max(nblk)
    # tile j is touched by requests [0, nbj[j]) (lengths sorted descending)
    nbj = [sum(1 for b in range(B) if nblk[b] > j) for j in range(jmax)]

    nc = bass.Bass()
    x_d = nc.dram_tensor("x", [B, D], F32, kind="ExternalInput")
    lnw_d = nc.dram_tensor("lnw", [P, NKC], F32, kind="ExternalInput")
    wq_d = nc.dram_tensor("wq", [D, NHL * HD], F32, kind="ExternalInput")
    wk_d = nc.dram_tensor("wk", [D, NHL * HD], F32, kind="ExternalInput")
    wv_d = nc.dram_tensor("wv", [D, NHL * HD], F32, kind="ExternalInput")
    wo_d = nc.dram_tensor("wo", [NHL * HD, D], F32, kind="ExternalInput")
    kc_d = nc.dram_tensor("kc", [B, T, NHL * HD], F32, kind="ExternalInput")
    vc_d = nc.dram_tensor("vc", [B, T, NHL * HD], F32, kind="ExternalInput")
    out_d = nc.dram_tensor("out", [B, D], F32, kind="ExternalOutput")

    with tile.TileContext(nc) as tc:
        with (
            tc.tile_pool(name="const", bufs=1) as const_pool,
            tc.tile_pool(name="persist", bufs=1) as persist,
            tc.tile_pool(name="pcols", bufs=1) as p_pool,
        ):
            identity = const_pool.tile([P, P], F32, tag="identity")
            make_identity(nc, identity[:])
            ones_col = const_pool.tile([P, 1], BF16, tag="ones")
            nc.gpsimd.memset(ones_col[:], 1.0)
            identity_bf = const_pool.tile([P, P], BF16, tag="idbf")
            make_identity(nc, identity_bf[:])
            zrow = const_pool.tile([1, 512], F32, tag="zrow")
            nc.gpsimd.memset(zrow[:], 0.0)
            lnw_sb = const_pool.tile([P, NKC], F32, tag="lnw")
            nc.sync.dma_start(lnw_sb[:], lnw_d[:, :])

            # ---------------- phase 1: RMSNorm + projections ----------------
            qt_sb = persist.tile([P, B * NHL], BF16, tag="qt")  # col h*16+b
            kt_sb = persist.tile([P, B * NHL], BF16, tag="kt")  # col h*16+b
            v_sb = persist.tile([B, NHL * HD], BF16, tag="vsb")
            xnt_sb = persist.tile([P, NKC * B], F32, tag="xnt")  # chunk kc at cols kc*16

            with (
                tc.tile_pool(name="p1sb", bufs=1) as p1sb,
                tc.tile_pool(name="wpool", bufs=3) as wpool,
                tc.tile_pool(name="p1ps", bufs=1, space="PSUM") as p1ps,
            ):
                x_sb = p1sb.tile([B, D], F32, tag="x")
                nc.sync.dma_start(x_sb[:], x_d[:, :])
                xsq = p1sb.tile([B, D], F32, tag="xsq")
                ssq = p1sb.tile([B, 1], F32, tag="ssq")
                nc.scalar.activation(
                    xsq[:], x_sb[:], mybir.ActivationFunctionType.Square,
                    accum_out=ssq[:],
                )
                ssq2 = p1sb.tile([B, 1], F32, tag="ssq2")
                nc.vector.tensor_scalar_add(ssq2[:], ssq[:], EPS * D)
                std = p1sb.tile([B, 1], F32, tag="std")
                nc.scalar.activation(
                    std[:], ssq2[:], mybir.ActivationFunctionType.Sqrt,
                    scale=1.0 / D,
                )
                rstd = p1sb.tile([B, 1], F32, tag="rstd")
                nc.vector.reciprocal(rstd[:], std[:])
                xn_sb = p1sb.tile([B, D], F32, tag="xn")
                nc.scalar.mul(xn_sb[:], x_sb[:], rstd[:])

                # transpose xn into 32 chunks [128, 16], scaled by ln_w chunk
                for kc in range(NKC):
                    xnt_ps = p1ps.tile([P, B], F32, tag="xntps", bufs=2)
                    nc.tensor.transpose(
                        xnt_ps[:],
                        xn_sb[:, kc * P : (kc + 1) * P],
                        identity[:B, :B],
                    )
                    nc.scalar.mul(
                        xnt_sb[:, kc * B : (kc + 1) * B], xnt_ps[:],
                        lnw_sb[:, kc : kc + 1],
                    )

                # projections in natural form [16, 512] (lhsT = xnt chunk,
                # 16-column weight loads) -- each accumulator is alone in its
                # bank so plain start/stop group semantics are safe
                q_ps = p1ps.tile([B, NHL * HD], F32, tag="qps", bufs=1)
                k_ps = p1ps.tile([B, NHL * HD], F32, tag="kps", bufs=1)
                v_ps = p1ps.tile([B, NHL * HD], F32, tag="vps", bufs=1)
                for kc in range(NKC):
                    wq_sb = wpool.tile([P, NHL * HD], F32, tag="wq")
                    nc.sync.dma_start(wq_sb[:], wq_d[kc * P : (kc + 1) * P, :])
                    wk_sb = wpool.tile([P, NHL * HD], F32, tag="wk")
                    nc.sync.dma_start(wk_sb[:], wk_d[kc * P : (kc + 1) * P, :])
                    wv_sb = wpool.tile([P, NHL * HD], F32, tag="wv")
                    nc.sync.dma_start(wv_sb[:], wv_d[kc * P : (kc + 1) * P, :])
                    xnt_c = xnt_sb[:, kc * B : (kc + 1) * B]
                    nc.tensor.matmul(
                        q_ps[:], xnt_c, wq_sb[:],
                        start=(kc == 0), stop=(kc == NKC - 1),
                    )
                    nc.tensor.matmul(
                        k_ps[:], xnt_c, wk_sb[:],
                        start=(kc == 0), stop=(kc == NKC - 1),
                    )
                    nc.tensor.matmul(
                        v_ps[:], xnt_c, wv_sb[:],
                        start=(kc == 0), stop=(kc == NKC - 1),
                    )
                q_nat = p1sb.tile([B, NHL * HD], F32, tag="qnat")
                nc.scalar.copy(q_nat[:], q_ps[:])
                k_nat = p1sb.tile([B, NHL * HD], F32, tag="knat")
                nc.scalar.copy(k_nat[:], k_ps[:])
                nc.scalar.copy(v_sb[:], v_ps[:])
                for h in range(NHL):
                    qh_ps = p1ps.tile([P, B], F32, tag="xntps", bufs=2)
                    nc.tensor.transpose(
                        qh_ps[:], q_nat[:, h * HD : (h + 1) * HD],
                        identity[:B, :B],
                    )
                    nc.scalar.copy(qt_sb[:, h * B : (h + 1) * B], qh_ps[:])
                    kh_ps = p1ps.tile([P, B], F32, tag="xntps", bufs=2)
                    nc.tensor.transpose(
                        kh_ps[:], k_nat[:, h * HD : (h + 1) * HD],
                        identity[:B, :B],
                    )
                    nc.scalar.copy(kt_sb[:, h * B : (h + 1) * B], kh_ps[:])

            # ---------------- pass K: scores ----------------
            sc_cm = tc.tile_pool(name="sc", bufs=1, space="PSUM")
            sc_pool = sc_cm.__enter__()
            zatt_cm = tc.tile_pool(name="zatt", bufs=1, space="PSUM")
            zatt_pool = zatt_cm.__enter__()
            # persistent score PSUM: 8 j-tiles of [128, 64] packed per bank
            n_sc_banks = (jmax + 7) // 8
            sc_ps = [
                sc_pool.tile([P, 512], F32, tag=f"sc{i}", name=f"sc{i}")
                for i in range(n_sc_banks)
            ]
            z_ps = zatt_pool.tile([1, B * NHL], F32, tag="z")
            attn_ps = zatt_pool.tile([P, B * NHL], F32, tag="attn")

            def sc_slice(j, c0, c1, p0, p1):
                return sc_ps[j // 8][p0:p1, (j % 8) * 64 + c0 : (j % 8) * 64 + c1]

            # memset all score banks to -1e30 (invalid rows stay masked)
            for t_ in sc_ps:
                nc.vector.memset(t_[:], -1.0e30)
            nc.tensor.matmul(
                z_ps[:], zrow[0:1, 0:1], zrow[0:1, : B * NHL],
                start=True, stop=True, skip_group_check=True,
            )
            nc.tensor.matmul(
                attn_ps[:], zrow[0:1, :P], zrow[0:1, : B * NHL],
                start=True, stop=True, skip_group_check=True,
            )

            with (
                tc.tile_pool(name="kpool", bufs=3) as kpool,
                tc.tile_pool(name="ktr", bufs=4) as ktr_pool,
                tc.tile_pool(name="ktrps", bufs=3, space="PSUM") as ktrps_pool,
            ):
                for b in range(B):
                    L = Ls[b]
                    ngrp = (nblk[b] + JG - 1) // JG
                    for jg in range(ngrp):
                        rows_g = max(0, min(JG * P, L - jg * JG * P))
                        q128, rem = divmod(rows_g, P)
                        if rows_g > 0:
                            # gpsimd (SWDGE) casts fp32->bf16 inline
                            k_tile = kpool.tile([P, JG * NHL * HD], BF16, tag="k")
                            if q128 > 0:
                                src = kc_d[
                                    b, jg * JG * P : jg * JG * P + q128 * P, :
                                ].rearrange("(jj p) d -> p jj d", p=P)
                                nc.gpsimd.dma_start(
                                    k_tile[:, : q128 * NHL * HD].rearrange(
                                        "p (jj d) -> p jj d", d=NHL * HD
                                    ),
                                    src,
                                )
                            if rem > 0:
                                nc.gpsimd.dma_start(
                                    k_tile[
                                        :rem,
                                        q128 * NHL * HD : (q128 + 1) * NHL * HD,
                                    ],
                                    kc_d[
                                        b,
                                        jg * JG * P + q128 * P : jg * JG * P + rows_g,
                                        :,
                                    ],
                                )
                        else:
                            k_tile = None
                        for jj in range(JG):
                            j = jg * JG + jj
                            if j >= nblk[b]:
                                break
                            tail = j == nblk[b] - 1
                            sub = min(P, max(0, L - j * P))  # cache rows here
                            m = vt[b] if tail else P        # scores to emit
                            for h in range(NHL):
                                ktr_sb = ktr_pool.tile([P, P], BF16, tag="ktr")
                                if sub > 0:
                                    # HW xbar transpose (bf16): rows padded to
                                    # a multiple of 16; the padded garbage
                                    # columns are never read (scores use :m,
                                    # and column r is overwritten by kT)
                                    tr = (sub + 15) // 16 * 16
                                    nc.sync.dma_start_transpose(
                                        ktr_sb[:, :tr],
                                        k_tile[
                                            :tr,
                                            jj * NHL * HD + h * HD : jj * NHL * HD + (h + 1) * HD,
                                        ],
                                    )
                                if tail:
                                    nc.vector.tensor_copy(
                                        ktr_sb[:, r[b] : r[b] + 1],
                                        kt_sb[:, h * B + b : h * B + b + 1],
                                    )
                                col = b * NHL + h
                                nc.tensor.matmul(
                                    sc_slice(j, col, col + 1, 0, m),
                                    ktr_sb[:, :m],
                                    qt_sb[:, h * B + b : h * B + b + 1],
                                    start=True, stop=True,
                                )

            # ---------------- softmax (no max-sub) ----------------
            p_cols = []
            for j in range(jmax):
                pc = p_pool.tile([P, B * NHL], BF16, tag=f"p{j}", name=f"p{j}")
                nc.scalar.activation(
                    pc[:, : NHL * nbj[j]],
                    sc_slice(j, 0, NHL * nbj[j], 0, P),
                    mybir.ActivationFunctionType.Exp,
                    scale=SCALE,
                )
                p_cols.append(pc)
            for j in range(jmax):
                nc.tensor.matmul(
                    z_ps[0:1, : NHL * nbj[j]],
                    ones_col[:],
                    p_cols[j][:, : NHL * nbj[j]],
                    start=False, stop=(j == jmax - 1),
                    skip_group_check=True,
                )
            invz_row = persist.tile([1, B * NHL], F32, tag="invzr")
            nc.vector.reciprocal(invz_row[:], z_ps[:])
            # bounce [1, 64] -> DRAM -> [16, 4] (free dim cannot become a
            # partition dim within SBUF without a transpose; DRAM is linear)
            invz_dram = nc.dram_tensor("invz_scratch", [1, B * NHL], F32)
            nc.gpsimd.dma_start(invz_dram[:, :], invz_row[:])
            invz_nat = persist.tile([B, NHL], F32, tag="invzn")
            nc.gpsimd.dma_start(
                invz_nat[:],
                invz_dram.rearrange("o (b h) -> (o b) h", b=B),
            )

            # ---------------- pass V: attn = p @ V ----------------
            with tc.tile_pool(name="vpool", bufs=3) as vpool:
                for b in range(B):
                    L = Ls[b]
                    ngrp = (nblk[b] + JG - 1) // JG
                    for jg in range(ngrp):
                        rows_g = max(0, min(JG * P, L - jg * JG * P))
                        q128, rem = divmod(rows_g, P)
                        v_tile = vpool.tile([P, JG * NHL * HD], BF16, tag="v")
                        if q128 > 0:
                            src = vc_d[
                                b, jg * JG * P : jg * JG * P + q128 * P, :
                            ].rearrange("(jj p) d -> p jj d", p=P)
                            nc.gpsimd.dma_start(
                                v_tile[:, : q128 * NHL * HD].rearrange(
                                    "p (jj d) -> p jj d", d=NHL * HD
                                ),
                                src,
                            )
                        if rem > 0:
                            nc.gpsimd.dma_start(
                                v_tile[
                                    :rem, q128 * NHL * HD : (q128 + 1) * NHL * HD
                                ],
                                vc_d[
                                    b,
                                    jg * JG * P + q128 * P : jg * JG * P + rows_g,
                                    :,
                                ],
                            )
                        for jj in range(JG):
                            j = jg * JG + jj
                            if j >= nblk[b]:
                                break
                            tail = j == nblk[b] - 1
                            m = vt[b] if tail else P
                            if tail:
                                # splice the new token's v row in
                                nc.gpsimd.dma_start(
                                    v_tile[
                                        r[b] : r[b] + 1,
                                        jj * NHL * HD : (jj + 1) * NHL * HD,
                                    ],
                                    v_sb[b : b + 1, :],
                                )
                            for h in range(NHL):
                                col = b * NHL + h
                                # attn columns are head-major so o_proj's lhsT
                                # per head is a contiguous [128, 16] slice
                                nc.tensor.matmul(
                                    attn_ps[:, h * B + b : h * B + b + 1],
                                    v_tile[
                                        :m,
                                        jj * NHL * HD + h * HD : jj * NHL * HD + (h + 1) * HD,
                                    ],
                                    p_cols[j][:m, col : col + 1],
                                    start=False, stop=tail,
                                    skip_group_check=True,
                                )

            attn_sb = persist.tile([P, B * NHL], F32, tag="attnsb")
            nc.scalar.copy(attn_sb[:], attn_ps[:])

            # ---------------- o_proj partial + 1/Z ----------------
            out_sb = persist.tile([B, D], F32, tag="outsb")
            with (
                tc.tile_pool(name="wopool", bufs=4) as wopool,
                tc.tile_pool(name="ops", bufs=4, space="PSUM") as o_ps_pool,
                tc.tile_pool(name="osb", bufs=2) as o_sb_pool,
            ):
                NCH = D // 512
                for nch in range(NCH):
                    scaled = []
                    for h in range(NHL):
                        wo_sb = wopool.tile([P, 512], F32, tag="wo")
                        nc.sync.dma_start(
                            wo_sb[:],
                            wo_d[h * HD : (h + 1) * HD, nch * 512 : (nch + 1) * 512],
                        )
                        o_ps = o_ps_pool.tile([B, 512], F32, tag="ops")
                        nc.tensor.matmul(
                            o_ps[:],
                            attn_sb[:, h * B : (h + 1) * B],
                            wo_sb[:],
                            start=True, stop=True,
                        )
                        s = o_sb_pool.tile([B, 512], F32, tag=f"os{h}")
                        nc.scalar.mul(s[:], o_ps[:], invz_nat[:, h : h + 1])
                        scaled.append(s)
                    acc = out_sb[:, nch * 512 : (nch + 1) * 512]
                    nc.vector.tensor_add(acc, scaled[0][:], scaled[1][:])
                    nc.vector.tensor_add(acc, acc, scaled[2][:])
                    nc.vector.tensor_add(acc, acc, scaled[3][:])
            nc.sync.dma_start(out_d[:, :], out_sb[:])
            zatt_cm.__exit__(None, None, None)
            sc_cm.__exit__(None, None, None)

    _split_excess_waits(nc)
    return nc


def _prep_inputs(x, ln_w, Wq, Wk, Wv, Wo, K_cache, V_cache, cache_lens):
    x = np.asarray(x, np.float32).reshape(B, D)
    ln_w = np.asarray(ln_w, np.float32)
    cache_lens = np.asarray(cache_lens, np.int32)
    perm = np.argsort(-cache_lens, kind="stable")
    Ls = [int(cache_lens[p]) for p in perm]
    lnw2d = np.ascontiguousarray(ln_w.reshape(NKC, P).T)
    x_s = np.ascontiguousarray(x[perm])
    K4 = np.asarray(K_cache, np.float32).reshape(B, T, H, HD)
    V4 = np.asarray(V_cache, np.float32).reshape(B, T, H, HD)
    in_maps = []
    for c in range(NCORES):
        h0 = c * NHL
        in_maps.append(
            {
                "x": x_s,
                "lnw": lnw2d,
                "wq": np.ascontiguousarray(
                    np.asarray(Wq, np.float32)[:, h0 * HD : (h0 + NHL) * HD]
                ),
                "wk": np.ascontiguousarray(
                    np.asarray(Wk, np.float32)[:, h0 * HD : (h0 + NHL) * HD]
                ),
                "wv": np.ascontiguousarray(
                    np.asarray(Wv, np.float32)[:, h0 * HD : (h0 + NHL) * HD]
                ),
                "wo": np.ascontiguousarray(
                    np.asarray(Wo, np.float32)[h0 * HD : (h0 + NHL) * HD, :]
                ),
                "kc": np.ascontiguousarray(
                    K4[perm][:, :, h0 : h0 + NHL, :]
                ).reshape(B, T, NHL * HD),
                "vc": np.ascontiguousarray(
                    V4[perm][:, :, h0 : h0 + NHL, :]
                ).reshape(B, T, NHL * HD),
            }
        )
    return in_maps, Ls, perm, x_s


def _run(x, ln_w, Wq, Wk, Wv, Wo, K_cache, V_cache, cache_lens, trace=False):
    in_maps, Ls, perm, x_s = _prep_inputs(
        x, ln_w, Wq, Wk, Wv, Wo, K_cache, V_cache, cache_lens
    )
    nc = _build(Ls)
    res = run_bass_kernel_spmd(
        nc, in_maps, core_ids=list(range(NCORES)), trace=trace
    )
    partial = np.zeros((B, D), np.float32)
    for c in range(NCORES):
        partial += res.results[c]["out"]
    out_sorted = x_s + partial
    out = np.empty((B, D), np.float32)
    out[perm] = out_sorted
    return out.reshape(B, 1, D), res


def kernel(x, ln_w, Wq, Wk, Wv, Wo, K_cache, V_cache, cache_lens):
    out, _ = _run(x, ln_w, Wq, Wk, Wv, Wo, K_cache, V_cache, cache_lens)
    return out


# revision 18
# speedup vs baseline: 3.5695x; 3.5695x over previous
"""Decode-step attention-partition kernel for 8 Trainium2 NeuronCores.

Shape (hardcoded from the problem spec):
  x[16,1,4096], ln_w[4096], Wq/Wk/Wv/Wo[4096,4096],
  K_cache/V_cache[16,2048,32,128], cache_lens[16] int32.

Sharding: head-parallel. Core c owns heads [4c, 4c+4) for ALL 16 requests:
  - RMSNorm of x replicated (tiny).
  - q/k computed directly in transposed form qT/kT [128d, 16b] per head;
    v in natural form [16, 512].
  - Ragged attention over the per-request cache with the new token folded in
    at position L_b (kT column spliced into the transposed K tile; v row
    spliced into the V tile). Softmax without max-subtraction (scores are
    O(10) here so exp() is safe in fp32); normalization 1/Z folded into the
    per-head o_proj epilogue as a per-partition scale.
  - Each core emits a partial o_proj [16, 4096] (its 4 heads' contribution);
    the host sums the 8 partials and adds the residual.

Request lengths are read on the host and baked into the instruction stream
(static trip counts, exact-size DMAs). Requests are sorted by length
descending so per-t-block "valid request" sets are prefixes.
"""

import sys
import types
import ctypes
import contextlib

import numpy as np

# ---------------------------------------------------------------------------
# axon NTFF profile hook (the image's antenv lacks axon_hooks; the capability
# exists in libaxon_pjrt.so). Registered before concourse.bass_utils import.
# ---------------------------------------------------------------------------


def _install_ntff_hook():
    if "antenv.axon_hooks" in sys.modules:
        return
    try:
        lib = ctypes.CDLL("/opt/axon/libaxon_pjrt.so")
        lib.axon_start_nrt_profile.argtypes = [
            ctypes.POINTER(ctypes.c_int64),
            ctypes.c_size_t,
        ]
        lib.axon_start_nrt_profile.restype = ctypes.c_int64
        lib.axon_stop_nrt_profile.argtypes = [ctypes.c_char_p]
        lib.axon_stop_nrt_profile.restype = ctypes.c_int64
    except OSError:
        lib = None

    @contextlib.contextmanager
    def _hook(output_dir, device_ids):
        import jax

        jax.devices()
        if device_ids:
            ids = (ctypes.c_int64 * len(device_ids))(*device_ids)
            rc = lib.axon_start_nrt_profile(ids, len(device_ids))
        else:
            rc = lib.axon_start_nrt_profile(None, 0)
        if rc != 0:
            raise RuntimeError(f"axon_start_nrt_profile rc={rc}")
        try:
            yield
        finally:
            n = lib.axon_stop_nrt_profile(str(output_dir).encode())
            print(f"ntff profile: {n} file(s) -> {output_dir}", file=sys.stderr)

    mod = types.ModuleType("antenv.axon_hooks")
    mod.get_axon_ntff_profile_hook = (lambda: _hook) if lib is not None else (lambda: None)
    mod.set_axon_ntff_profile_hook = lambda h: None
    sys.modules["antenv.axon_hooks"] = mod


_install_ntff_hook()

import concourse.bass as bass
import concourse.mybir as mybir
import concourse.tile as tile
from concourse.vector_clock import ScopedClock
from concourse.masks import make_identity
from concourse.bass_utils import run_bass_kernel_spmd

# ---------------------------------------------------------------------------
# This walrus build rejects instructions with >1 semaphore wait command
# ("Too many sync wait commands" in setupSyncWait for CTRL structs). Tile's
# kernel-tail drain accumulates one wait per engine/DMA lane. Split the waits
# across preceding same-engine NOPs (1 wait each).
# ---------------------------------------------------------------------------
_MAXW = 1


def _patched_drain_and_barrier(self, tick_clock, wait_clock):
    nc = self.nc
    probe = nc.sync.nop(nofuse=True)
    wait_clock.add_sem_waits(probe.ins, ScopedClock({None: tick_clock.global_clock}))
    si = probe.ins.sync_info
    waits = list(si.on_wait) if si is not None else []
    if len(waits) > _MAXW:
        si.on_wait = waits[:_MAXW]
        for i in range(_MAXW, len(waits), _MAXW):
            nop = nc.sync.nop(nofuse=True)
            nop.ins.sync_info = mybir.SyncInfo(
                on_wait=waits[i : i + _MAXW], on_update=[]
            )
    nc.sync.drain()
    nc.all_engine_barrier()
    assert self.sems is not None
    popped = nc._tile_sem_poison_stack.pop()
    assert popped is self._sem_poison
    nc.clear_and_free_semaphores(list(self.sems.allocated().values()))
    nc.all_engine_barrier()


tile.TileContext._drain_and_barrier = _patched_drain_and_barrier

_wsplit_counter = [0]


def _split_excess_waits(nc):
    """Post-pass: this walrus build allows at most 1 sem-wait per instruction.
    Move excess waits onto preceding same-engine NoOps (same-engine program
    order preserves the wait semantics)."""
    for fn in nc.m.functions:
        for bb in fn.blocks:
            out = []
            changed = False
            for inst in bb.instructions:
                si = inst.sync_info
                if (
                    si is not None
                    and len(si.on_wait) > 1
                    and not isinstance(inst, mybir.InstAllEngineBarrier)
                ):
                    waits = list(si.on_wait)
                    for w in waits[:-1]:
                        _wsplit_counter[0] += 1
                        out.append(
                            mybir.InstNoOp(
                                name=f"I-wsplit-{_wsplit_counter[0]}",
                                engine=inst.engine,
                                sync_info=mybir.SyncInfo(
                                    on_wait=[w], on_update=[]
                                ),
                            )
                        )
                    si.on_wait = [waits[-1]]
                    changed = True
                out.append(inst)
            if changed:
                bb.instructions[:] = out

# ---------------------------------------------------------------------------

F32 = mybir.dt.float32
BF16 = mybir.dt.bfloat16
P = 128
B = 16
T = 2048
D = 4096
H = 32
HD = 128
NHL = 4          # heads per core
NCORES = 8
EPS = 1e-6
NKC = D // P     # 32 contraction chunks for the projections
SCALE = 1.0 / float(np.sqrt(HD))
JG = 4           # t-blocks per K/V DMA group (4*128 rows x 2KB = 1 MiB)


def _build(Ls):
    """Build the per-core Bass kernel. Ls: 16 request lengths, sorted desc."""
    nblk = [l // P + 1 for l in Ls]          # t-blocks incl. the new token
    r = [l % P for l in Ls]                  # new-token row within tail block
    vt = [rr + 1 for rr in r]                # valid rows in tail block
    jmax = max(nblk)
    # tile j is touched by requests [0, nbj[j]) (lengths sorted descending)
    nbj = [sum(1 for b in range(B) if nblk[b] > j) for j in range(jmax)]

    nc = bass.Bass()
    x_d = nc.dram_tensor("x", [B, D], F32, kind="ExternalInput")
    lnw_d = nc.dram_tensor("lnw", [P, NKC], F32, kind="ExternalInput")
    wq_d = nc.dram_tensor("wq", [D, NHL * HD], F32, kind="ExternalInput")
    wk_d = nc.dram_tensor("wk", [D, NHL * HD], F32, kind="ExternalInput")
    wv_d = nc.dram_tensor("wv", [D, NHL * HD], F32, kind="ExternalInput")
    wo_d = nc.dram_tensor("wo", [NHL * HD, D], F32, kind="ExternalInput")
    kc_d = nc.dram_tensor("kc", [B, T, NHL * HD], F32, kind="ExternalInput")
    vc_d = nc.dram_tensor("vc", [B, T, NHL * HD], F32, kind="ExternalInput")
    out_d = nc.dram_tensor("out", [B, D], F32, kind="ExternalOutput")

    with tile.TileContext(nc) as tc:
        with (
            tc.tile_pool(name="const", bufs=1) as const_pool,
            tc.tile_pool(name="persist", bufs=1) as persist,
            tc.tile_pool(name="pcols", bufs=1) as p_pool,
        ):
            identity = const_pool.tile([P, P], F32, tag="identity")
            make_identity(nc, identity[:])
            ones_col = const_pool.tile([P, 1], BF16, tag="ones")
            nc.gpsimd.memset(ones_col[:], 1.0)
            identity_bf = const_pool.tile([P, P], BF16, tag="idbf")
            make_identity(nc, identity_bf[:])
            zrow = const_pool.tile([1, 512], F32, tag="zrow")
            nc.gpsimd.memset(zrow[:], 0.0)
            lnw_sb = const_pool.tile([P, NKC], F32, tag="lnw")
            nc.sync.dma_start(lnw_sb[:], lnw_d[:, :])

            # ---------------- phase 1: RMSNorm + projections ----------------
            qt_sb = persist.tile([P, B * NHL], BF16, tag="qt")  # col h*16+b
            kt_sb = persist.tile([P, B * NHL], BF16, tag="kt")  # col h*16+b
            v_sb = persist.tile([B, NHL * HD], BF16, tag="vsb")
            xnt_sb = persist.tile([P, NKC * B], BF16, tag="xnt")  # chunk kc at cols kc*16

            with (
                tc.tile_pool(name="p1sb", bufs=1) as p1sb,
                tc.tile_pool(name="wpool", bufs=3) as wpool,
                tc.tile_pool(name="p1ps", bufs=1, space="PSUM") as p1ps,
            ):
                x_sb = p1sb.tile([B, D], F32, tag="x")
                nc.sync.dma_start(x_sb[:], x_d[:, :])
                xsq = p1sb.tile([B, D], F32, tag="xsq")
                ssq = p1sb.tile([B, 1], F32, tag="ssq")
                nc.scalar.activation(
                    xsq[:], x_sb[:], mybir.ActivationFunctionType.Square,
                    accum_out=ssq[:],
                )
                ssq2 = p1sb.tile([B, 1], F32, tag="ssq2")
                nc.vector.tensor_scalar_add(ssq2[:], ssq[:], EPS * D)
                std = p1sb.tile([B, 1], F32, tag="std")
                nc.scalar.activation(
                    std[:], ssq2[:], mybir.ActivationFunctionType.Sqrt,
                    scale=1.0 / D,
                )
                rstd = p1sb.tile([B, 1], F32, tag="rstd")
                nc.vector.reciprocal(rstd[:], std[:])
                xn_sb = p1sb.tile([B, D], F32, tag="xn")
                nc.scalar.mul(xn_sb[:], x_sb[:], rstd[:])

                # transpose xn into 32 chunks [128, 16], scaled by ln_w chunk
                for kc in range(NKC):
                    xnt_ps = p1ps.tile([P, B], F32, tag="xntps", bufs=2)
                    nc.tensor.transpose(
                        xnt_ps[:],
                        xn_sb[:, kc * P : (kc + 1) * P],
                        identity[:B, :B],
                    )
                    nc.scalar.mul(
                        xnt_sb[:, kc * B : (kc + 1) * B], xnt_ps[:],
                        lnw_sb[:, kc : kc + 1],
                    )

                # projections in natural form [16, 512] (lhsT = xnt chunk,
                # 16-column weight loads) -- each accumulator is alone in its
                # bank so plain start/stop group semantics are safe
                q_ps = p1ps.tile([B, NHL * HD], F32, tag="qps", bufs=1)
                k_ps = p1ps.tile([B, NHL * HD], F32, tag="kps", bufs=1)
                v_ps = p1ps.tile([B, NHL * HD], F32, tag="vps", bufs=1)
                for kc in range(NKC):
                    wq_sb = wpool.tile([P, NHL * HD], BF16, tag="wq")
                    nc.gpsimd.dma_start(wq_sb[:], wq_d[kc * P : (kc + 1) * P, :])
                    wk_sb = wpool.tile([P, NHL * HD], BF16, tag="wk")
                    nc.gpsimd.dma_start(wk_sb[:], wk_d[kc * P : (kc + 1) * P, :])
                    wv_sb = wpool.tile([P, NHL * HD], BF16, tag="wv")
                    nc.gpsimd.dma_start(wv_sb[:], wv_d[kc * P : (kc + 1) * P, :])
                    xnt_c = xnt_sb[:, kc * B : (kc + 1) * B]
                    nc.tensor.matmul(
                        q_ps[:], xnt_c, wq_sb[:],
                        start=(kc == 0), stop=(kc == NKC - 1),
                    )
                    nc.tensor.matmul(
                        k_ps[:], xnt_c, wk_sb[:],
                        start=(kc == 0), stop=(kc == NKC - 1),
                    )
                    nc.tensor.matmul(
                        v_ps[:], xnt_c, wv_sb[:],
                        start=(kc == 0), stop=(kc == NKC - 1),
                    )
                q_nat = p1sb.tile([B, NHL * HD], F32, tag="qnat")
                nc.scalar.copy(q_nat[:], q_ps[:])
                k_nat = p1sb.tile([B, NHL * HD], F32, tag="knat")
                nc.scalar.copy(k_nat[:], k_ps[:])
                nc.scalar.copy(v_sb[:], v_ps[:])
                for h in range(NHL):
                    qh_ps = p1ps.tile([P, B], F32, tag="xntps", bufs=2)
                    nc.tensor.transpose(
                        qh_ps[:], q_nat[:, h * HD : (h + 1) * HD],
                        identity[:B, :B],
                    )
                    nc.scalar.copy(qt_sb[:, h * B : (h + 1) * B], qh_ps[:])
                    kh_ps = p1ps.tile([P, B], F32, tag="xntps", bufs=2)
                    nc.tensor.transpose(
                        kh_ps[:], k_nat[:, h * HD : (h + 1) * HD],
                        identity[:B, :B],
                    )
                    nc.scalar.copy(kt_sb[:, h * B : (h + 1) * B], kh_ps[:])

            # ---------------- pass K: scores ----------------
            sc_cm = tc.tile_pool(name="sc", bufs=1, space="PSUM")
            sc_pool = sc_cm.__enter__()
            zatt_cm = tc.tile_pool(name="zatt", bufs=1, space="PSUM")
            zatt_pool = zatt_cm.__enter__()
            # persistent score PSUM: 8 j-tiles of [128, 64] packed per bank
            n_sc_banks = (jmax + 7) // 8
            sc_ps = [
                sc_pool.tile([P, 512], F32, tag=f"sc{i}", name=f"sc{i}")
                for i in range(n_sc_banks)
            ]
            z_ps = zatt_pool.tile([1, B * NHL], F32, tag="z")
            attn_ps = zatt_pool.tile([P, B * NHL], F32, tag="attn")

            def sc_slice(j, c0, c1, p0, p1):
                return sc_ps[j // 8][p0:p1, (j % 8) * 64 + c0 : (j % 8) * 64 + c1]

            # memset all score banks to -1e30 (invalid rows stay masked)
            for t_ in sc_ps:
                nc.vector.memset(t_[:], -1.0e30)
            nc.tensor.matmul(
                z_ps[:], zrow[0:1, 0:1], zrow[0:1, : B * NHL],
                start=True, stop=True, skip_group_check=True,
            )
            nc.tensor.matmul(
                attn_ps[:], zrow[0:1, :P], zrow[0:1, : B * NHL],
                start=True, stop=True, skip_group_check=True,
            )

            with (
                tc.tile_pool(name="kpool", bufs=6) as kpool,
                tc.tile_pool(name="ktr", bufs=8) as ktr_pool,
                tc.tile_pool(name="ktrps", bufs=4, space="PSUM") as ktrps_pool,
            ):
                for b in range(B):
                    L = Ls[b]
                    ngrp = (nblk[b] + JG - 1) // JG
                    for jg in range(ngrp):
                        rows_g = max(0, min(JG * P, L - jg * JG * P))
                        q128, rem = divmod(rows_g, P)
                        if rows_g > 0:
                            # gpsimd (SWDGE) casts fp32->bf16 inline
                            k_tile = kpool.tile([P, JG * NHL * HD], BF16, tag="k")
                            if q128 > 0:
                                src = kc_d[
                                    b, jg * JG * P : jg * JG * P + q128 * P, :
                                ].rearrange("(jj p) d -> p jj d", p=P)
                                nc.gpsimd.dma_start(
                                    k_tile[:, : q128 * NHL * HD].rearrange(
                                        "p (jj d) -> p jj d", d=NHL * HD
                                    ),
                                    src,
                                )
                            if rem > 0:
                                nc.gpsimd.dma_start(
                                    k_tile[
                                        :rem,
                                        q128 * NHL * HD : (q128 + 1) * NHL * HD,
                                    ],
                                    kc_d[
                                        b,
                                        jg * JG * P + q128 * P : jg * JG * P + rows_g,
                                        :,
                                    ],
                                )
                        else:
                            k_tile = None
                        for jj in range(JG):
                            j = jg * JG + jj
                            if j >= nblk[b]:
                                break
                            tail = j == nblk[b] - 1
                            sub = min(P, max(0, L - j * P))  # cache rows here
                            m = vt[b] if tail else P        # scores to emit
                            for h in range(NHL):
                                ktr_sb = ktr_pool.tile([P, P], BF16, tag="ktr")
                                if sub > 0:
                                    ktr_ps = ktrps_pool.tile([P, P], BF16, tag="ktrp")
                                    nc.tensor.transpose(
                                        ktr_ps[:, :sub],
                                        k_tile[
                                            :sub,
                                            jj * NHL * HD + h * HD : jj * NHL * HD + (h + 1) * HD,
                                        ],
                                        identity_bf[:sub, :sub],
                                    )
                                    # alternate evacuation between ACT and DVE
                                    if (b + j + h) % 2 == 0:
                                        nc.scalar.copy(
                                            ktr_sb[:, :sub], ktr_ps[:, :sub]
                                        )
                                    else:
                                        nc.vector.tensor_copy(
                                            ktr_sb[:, :sub], ktr_ps[:, :sub]
                                        )
                                if tail:
                                    nc.vector.tensor_copy(
                                        ktr_sb[:, r[b] : r[b] + 1],
                                        kt_sb[:, h * B + b : h * B + b + 1],
                                    )
                                col = b * NHL + h
                                nc.tensor.matmul(
                                    sc_slice(j, col, col + 1, 0, m),
                                    ktr_sb[:, :m],
                                    qt_sb[:, h * B + b : h * B + b + 1],
                                    start=True, stop=True,
                                )

            # ---------------- softmax (no max-sub) ----------------
            p_cols = []
            for j in range(jmax):
                pc = p_pool.tile([P, B * NHL], BF16, tag=f"p{j}", name=f"p{j}")
                nc.scalar.activation(
                    pc[:, : NHL * nbj[j]],
                    sc_slice(j, 0, NHL * nbj[j], 0, P),
                    mybir.ActivationFunctionType.Exp,
                    scale=SCALE,
                )
                p_cols.append(pc)
            for j in range(jmax):
                nc.tensor.matmul(
                    z_ps[0:1, : NHL * nbj[j]],
                    ones_col[:],
                    p_cols[j][:, : NHL * nbj[j]],
                    start=False, stop=(j == jmax - 1),
                    skip_group_check=True,
                )
            invz_row = persist.tile([1, B * NHL], F32, tag="invzr")
            nc.vector.reciprocal(invz_row[:], z_ps[:])
            # bounce [1, 64] -> DRAM -> [16, 4] (free dim cannot become a
            # partition dim within SBUF without a transpose; DRAM is linear)
            invz_dram = nc.dram_tensor("invz_scratch", [1, B * NHL], F32)
            nc.gpsimd.dma_start(invz_dram[:, :], invz_row[:])
            invz_nat = persist.tile([B, NHL], F32, tag="invzn")
            nc.gpsimd.dma_start(
                invz_nat[:],
                invz_dram.rearrange("o (b h) -> (o b) h", b=B),
            )

            # ---------------- pass V: attn = p @ V ----------------
            with tc.tile_pool(name="vpool", bufs=6) as vpool:
                for b in range(B):
                    L = Ls[b]
                    ngrp = (nblk[b] + JG - 1) // JG
                    for jg in range(ngrp):
                        rows_g = max(0, min(JG * P, L - jg * JG * P))
                        q128, rem = divmod(rows_g, P)
                        v_tile = vpool.tile([P, JG * NHL * HD], BF16, tag="v")
                        if q128 > 0:
                            src = vc_d[
                                b, jg * JG * P : jg * JG * P + q128 * P, :
                            ].rearrange("(jj p) d -> p jj d", p=P)
                            nc.gpsimd.dma_start(
                                v_tile[:, : q128 * NHL * HD].rearrange(
                                    "p (jj d) -> p jj d", d=NHL * HD
                                ),
                                src,
                            )
                        if rem > 0:
                            nc.gpsimd.dma_start(
                                v_tile[
                                    :rem, q128 * NHL * HD : (q128 + 1) * NHL * HD
                                ],
                                vc_d[
                                    b,
                                    jg * JG * P + q128 * P : jg * JG * P + rows_g,
                                    :,
                                ],
                            )
                        for jj in range(JG):
                            j = jg * JG + jj
                            if j >= nblk[b]:
                                break
                            tail = j == nblk[b] - 1
                            m = vt[b] if tail else P
                            if tail:
                                # splice the new token's v row in
                                nc.gpsimd.dma_start(
                                    v_tile[
                                        r[b] : r[b] + 1,
                                        jj * NHL * HD : (jj + 1) * NHL * HD,
                                    ],
                                    v_sb[b : b + 1, :],
                                )
                            for h in range(NHL):
                                col = b * NHL + h
                                # attn columns are head-major so o_proj's lhsT
                                # per head is a contiguous [128, 16] slice
                                nc.tensor.matmul(
                                    attn_ps[:, h * B + b : h * B + b + 1],
                                    v_tile[
                                        :m,
                                        jj * NHL * HD + h * HD : jj * NHL * HD + (h + 1) * HD,
                                    ],
                                    p_cols[j][:m, col : col + 1],
                                    start=False, stop=tail,
                                    skip_group_check=True,
                                )

            attn_sb = persist.tile([P, B * NHL], BF16, tag="attnsb")
            nc.scalar.copy(attn_sb[:], attn_ps[:])

            # ---------------- o_proj partial + 1/Z ----------------
            out_sb = persist.tile([B, D], F32, tag="outsb")
            with (
                tc.tile_pool(name="wopool", bufs=4) as wopool,
                tc.tile_pool(name="ops", bufs=4, space="PSUM") as o_ps_pool,
                tc.tile_pool(name="osb", bufs=2) as o_sb_pool,
            ):
                NCH = D // 512
                for nch in range(NCH):
                    scaled = []
                    for h in range(NHL):
                        wo_sb = wopool.tile([P, 512], BF16, tag="wo")
                        nc.gpsimd.dma_start(
                            wo_sb[:],
                            wo_d[h * HD : (h + 1) * HD, nch * 512 : (nch + 1) * 512],
                        )
                        o_ps = o_ps_pool.tile([B, 512], F32, tag="ops")
                        nc.tensor.matmul(
                            o_ps[:],
                            attn_sb[:, h * B : (h + 1) * B],
                            wo_sb[:],
                            start=True, stop=True,
                        )
                        s = o_sb_pool.tile([B, 512], F32, tag=f"os{h}")
                        nc.scalar.mul(s[:], o_ps[:], invz_nat[:, h : h + 1])
                        scaled.append(s)
                    acc = out_sb[:, nch * 512 : (nch + 1) * 512]
                    nc.vector.tensor_add(acc, scaled[0][:], scaled[1][:])
                    nc.vector.tensor_add(acc, acc, scaled[2][:])
                    nc.vector.tensor_add(acc, acc, scaled[3][:])
            nc.sync.dma_start(out_d[:, :], out_sb[:])
            zatt_cm.__exit__(None, None, None)
            sc_cm.__exit__(None, None, None)

    _split_excess_waits(nc)
    return nc


def _prep_inputs(x, ln_w, Wq, Wk, Wv, Wo, K_cache, V_cache, cache_lens):
    x = np.asarray(x, np.float32).reshape(B, D)
    ln_w = np.asarray(ln_w, np.float32)
    cache_lens = np.asarray(cache_lens, np.int32)
    perm = np.argsort(-cache_lens, kind="stable")
    Ls = [int(cache_lens[p]) for p in perm]
    lnw2d = np.ascontiguousarray(ln_w.reshape(NKC, P).T)
    x_s = np.ascontiguousarray(x[perm])
    K4 = np.asarray(K_cache, np.float32).reshape(B, T, H, HD)
    V4 = np.asarray(V_cache, np.float32).reshape(B, T, H, HD)
    in_maps = []
    for c in range(NCORES):
        h0 = c * NHL
        in_maps.append(
            {
                "x": x_s,
                "lnw": lnw2d,
                "wq": np.ascontiguousarray(
                    np.asarray(Wq, np.float32)[:, h0 * HD : (h0 + NHL) * HD]
                ),
                "wk": np.ascontiguousarray(
                    np.asarray(Wk, np.float32)[:, h0 * HD : (h0 + NHL) * HD]
                ),
                "wv": np.ascontiguousarray(
                    np.asarray(Wv, np.float32)[:, h0 * HD : (h0 + NHL) * HD]
                ),
                "wo": np.ascontiguousarray(
                    np.asarray(Wo, np.float32)[h0 * HD : (h0 + NHL) * HD, :]
                ),
                "kc": np.ascontiguousarray(
                    K4[perm][:, :, h0 : h0 + NHL, :]
                ).reshape(B, T, NHL * HD),
                "vc": np.ascontiguousarray(
                    V4[perm][:, :, h0 : h0 + NHL, :]
                ).reshape(B, T, NHL * HD),
            }
        )
    return in_maps, Ls, perm, x_s


def _run(x, ln_w, Wq, Wk, Wv, Wo, K_cache, V_cache, cache_lens, trace=False):
    in_maps, Ls, perm, x_s = _prep_inputs(
        x, ln_w, Wq, Wk, Wv, Wo, K_cache, V_cache, cache_lens
    )
    nc = _build(Ls)
    res = run_bass_kernel_spmd(
        nc, in_maps, core_ids=list(range(NCORES)), trace=trace
    )
    partial = np.zeros((B, D), np.float32)
    for c in range(NCORES):
        partial += res.results[c]["out"]
    out_sorted = x_s + partial
    out = np.empty((B, D), np.float32)
    out[perm] = out_sorted
    return out.reshape(B, 1, D), res


def kernel(x, ln_w, Wq, Wk, Wv, Wo, K_cache, V_cache, cache_lens):
    out, _ = _run(x, ln_w, Wq, Wk, Wv, Wo, K_cache, V_cache, cache_lens)
    return out


# revision 19
# speedup vs baseline: 4.3347x; 1.2144x over previous
"""Decode-step attention-partition kernel for 8 Trainium2 NeuronCores.

Shape (hardcoded from the problem spec):
  x[16,1,4096], ln_w[4096], Wq/Wk/Wv/Wo[4096,4096],
  K_cache/V_cache[16,2048,32,128], cache_lens[16] int32.

Sharding: head-parallel. Core c owns heads [4c, 4c+4) for ALL 16 requests:
  - RMSNorm of x replicated (tiny).
  - q/k computed directly in transposed form qT/kT [128d, 16b] per head;
    v in natural form [16, 512].
  - Ragged attention over the per-request cache with the new token folded in
    at position L_b (kT column spliced into the transposed K tile; v row
    spliced into the V tile). Softmax without max-subtraction (scores are
    O(10) here so exp() is safe in fp32); normalization 1/Z folded into the
    per-head o_proj epilogue as a per-partition scale.
  - Each core emits a partial o_proj [16, 4096] (its 4 heads' contribution);
    the host sums the 8 partials and adds the residual.

Request lengths are read on the host and baked into the instruction stream
(static trip counts, exact-size DMAs). Requests are sorted by length
descending so per-t-block "valid request" sets are prefixes.
"""

import sys
import types
import ctypes
import contextlib

import numpy as np
import ml_dtypes

BF16_NP = ml_dtypes.bfloat16

# ---------------------------------------------------------------------------
# axon NTFF profile hook (the image's antenv lacks axon_hooks; the capability
# exists in libaxon_pjrt.so). Registered before concourse.bass_utils import.
# ---------------------------------------------------------------------------


def _install_ntff_hook():
    if "antenv.axon_hooks" in sys.modules:
        return
    try:
        lib = ctypes.CDLL("/opt/axon/libaxon_pjrt.so")
        lib.axon_start_nrt_profile.argtypes = [
            ctypes.POINTER(ctypes.c_int64),
            ctypes.c_size_t,
        ]
        lib.axon_start_nrt_profile.restype = ctypes.c_int64
        lib.axon_stop_nrt_profile.argtypes = [ctypes.c_char_p]
        lib.axon_stop_nrt_profile.restype = ctypes.c_int64
    except OSError:
        lib = None

    @contextlib.contextmanager
    def _hook(output_dir, device_ids):
        import jax

        jax.devices()
        if device_ids:
            ids = (ctypes.c_int64 * len(device_ids))(*device_ids)
            rc = lib.axon_start_nrt_profile(ids, len(device_ids))
        else:
            rc = lib.axon_start_nrt_profile(None, 0)
        if rc != 0:
            raise RuntimeError(f"axon_start_nrt_profile rc={rc}")
        try:
            yield
        finally:
            n = lib.axon_stop_nrt_profile(str(output_dir).encode())
            print(f"ntff profile: {n} file(s) -> {output_dir}", file=sys.stderr)

    mod = types.ModuleType("antenv.axon_hooks")
    mod.get_axon_ntff_profile_hook = (lambda: _hook) if lib is not None else (lambda: None)
    mod.set_axon_ntff_profile_hook = lambda h: None
    sys.modules["antenv.axon_hooks"] = mod


_install_ntff_hook()

import concourse.bass as bass
import concourse.mybir as mybir
import concourse.tile as tile
from concourse.vector_clock import ScopedClock
from concourse.masks import make_identity
from concourse.bass_utils import run_bass_kernel_spmd

# ---------------------------------------------------------------------------
# This walrus build rejects instructions with >1 semaphore wait command
# ("Too many sync wait commands" in setupSyncWait for CTRL structs). Tile's
# kernel-tail drain accumulates one wait per engine/DMA lane. Split the waits
# across preceding same-engine NOPs (1 wait each).
# ---------------------------------------------------------------------------
_MAXW = 1


def _patched_drain_and_barrier(self, tick_clock, wait_clock):
    nc = self.nc
    probe = nc.sync.nop(nofuse=True)
    wait_clock.add_sem_waits(probe.ins, ScopedClock({None: tick_clock.global_clock}))
    si = probe.ins.sync_info
    waits = list(si.on_wait) if si is not None else []
    if len(waits) > _MAXW:
        si.on_wait = waits[:_MAXW]
        for i in range(_MAXW, len(waits), _MAXW):
            nop = nc.sync.nop(nofuse=True)
            nop.ins.sync_info = mybir.SyncInfo(
                on_wait=waits[i : i + _MAXW], on_update=[]
            )
    nc.sync.drain()
    nc.all_engine_barrier()
    assert self.sems is not None
    popped = nc._tile_sem_poison_stack.pop()
    assert popped is self._sem_poison
    nc.clear_and_free_semaphores(list(self.sems.allocated().values()))
    nc.all_engine_barrier()


tile.TileContext._drain_and_barrier = _patched_drain_and_barrier

_wsplit_counter = [0]


def _split_excess_waits(nc):
    """Post-pass: this walrus build allows at most 1 sem-wait per instruction.
    Move excess waits onto preceding same-engine NoOps (same-engine program
    order preserves the wait semantics)."""
    for fn in nc.m.functions:
        for bb in fn.blocks:
            out = []
            changed = False
            for inst in bb.instructions:
                si = inst.sync_info
                if (
                    si is not None
                    and len(si.on_wait) > 1
                    and not isinstance(inst, mybir.InstAllEngineBarrier)
                ):
                    waits = list(si.on_wait)
                    for w in waits[:-1]:
                        _wsplit_counter[0] += 1
                        out.append(
                            mybir.InstNoOp(
                                name=f"I-wsplit-{_wsplit_counter[0]}",
                                engine=inst.engine,
                                sync_info=mybir.SyncInfo(
                                    on_wait=[w], on_update=[]
                                ),
                            )
                        )
                    si.on_wait = [waits[-1]]
                    changed = True
                out.append(inst)
            if changed:
                bb.instructions[:] = out

# ---------------------------------------------------------------------------

F32 = mybir.dt.float32
BF16 = mybir.dt.bfloat16
P = 128
B = 16
T = 2048
D = 4096
H = 32
HD = 128
NHL = 4          # heads per core
NCORES = 8
EPS = 1e-6
NKC = D // P     # 32 contraction chunks for the projections
SCALE = 1.0 / float(np.sqrt(HD))
JG = 4           # t-blocks per K/V DMA group (4*128 rows x 2KB = 1 MiB)


def _build(Ls):
    """Build the per-core Bass kernel. Ls: 16 request lengths, sorted desc."""
    nblk = [l // P + 1 for l in Ls]          # t-blocks incl. the new token
    r = [l % P for l in Ls]                  # new-token row within tail block
    vt = [rr + 1 for rr in r]                # valid rows in tail block
    jmax = max(nblk)
    # tile j is touched by requests [0, nbj[j]) (lengths sorted descending)
    nbj = [sum(1 for b in range(B) if nblk[b] > j) for j in range(jmax)]

    nc = bass.Bass()
    x_d = nc.dram_tensor("x", [B, D], F32, kind="ExternalInput")
    lnw_d = nc.dram_tensor("lnw", [P, NKC], F32, kind="ExternalInput")
    wq_d = nc.dram_tensor("wq", [D, NHL * HD], BF16, kind="ExternalInput")
    wk_d = nc.dram_tensor("wk", [D, NHL * HD], BF16, kind="ExternalInput")
    wv_d = nc.dram_tensor("wv", [D, NHL * HD], BF16, kind="ExternalInput")
    wo_d = nc.dram_tensor("wo", [NHL * HD, D], BF16, kind="ExternalInput")
    kc_d = nc.dram_tensor("kc", [B, T, NHL * HD], BF16, kind="ExternalInput")
    vc_d = nc.dram_tensor("vc", [B, T, NHL * HD], BF16, kind="ExternalInput")
    out_d = nc.dram_tensor("out", [B, D], F32, kind="ExternalOutput")

    with tile.TileContext(nc) as tc:
        with (
            tc.tile_pool(name="const", bufs=1) as const_pool,
            tc.tile_pool(name="persist", bufs=1) as persist,
            tc.tile_pool(name="pcols", bufs=1) as p_pool,
        ):
            identity = const_pool.tile([P, P], F32, tag="identity")
            make_identity(nc, identity[:])
            ones_col = const_pool.tile([P, 1], BF16, tag="ones")
            nc.gpsimd.memset(ones_col[:], 1.0)
            identity_bf = const_pool.tile([P, P], BF16, tag="idbf")
            make_identity(nc, identity_bf[:])
            zrow = const_pool.tile([1, 512], F32, tag="zrow")
            nc.gpsimd.memset(zrow[:], 0.0)
            lnw_sb = const_pool.tile([P, NKC], F32, tag="lnw")
            nc.sync.dma_start(lnw_sb[:], lnw_d[:, :])

            # ---------------- phase 1: RMSNorm + projections ----------------
            qt_sb = persist.tile([P, B * NHL], BF16, tag="qt")  # col h*16+b
            kt_sb = persist.tile([P, B * NHL], BF16, tag="kt")  # col h*16+b
            v_sb = persist.tile([B, NHL * HD], BF16, tag="vsb")
            xnt_sb = persist.tile([P, NKC * B], BF16, tag="xnt")  # chunk kc at cols kc*16

            with (
                tc.tile_pool(name="p1sb", bufs=1) as p1sb,
                tc.tile_pool(name="wpool", bufs=3) as wpool,
                tc.tile_pool(name="p1ps", bufs=1, space="PSUM") as p1ps,
            ):
                x_sb = p1sb.tile([B, D], F32, tag="x")
                nc.sync.dma_start(x_sb[:], x_d[:, :])
                xsq = p1sb.tile([B, D], F32, tag="xsq")
                ssq = p1sb.tile([B, 1], F32, tag="ssq")
                nc.scalar.activation(
                    xsq[:], x_sb[:], mybir.ActivationFunctionType.Square,
                    accum_out=ssq[:],
                )
                ssq2 = p1sb.tile([B, 1], F32, tag="ssq2")
                nc.vector.tensor_scalar_add(ssq2[:], ssq[:], EPS * D)
                std = p1sb.tile([B, 1], F32, tag="std")
                nc.scalar.activation(
                    std[:], ssq2[:], mybir.ActivationFunctionType.Sqrt,
                    scale=1.0 / D,
                )
                rstd = p1sb.tile([B, 1], F32, tag="rstd")
                nc.vector.reciprocal(rstd[:], std[:])
                xn_sb = p1sb.tile([B, D], F32, tag="xn")
                nc.scalar.mul(xn_sb[:], x_sb[:], rstd[:])

                # transpose xn into 32 chunks [128, 16], scaled by ln_w chunk
                for kc in range(NKC):
                    xnt_ps = p1ps.tile([P, B], F32, tag="xntps", bufs=2)
                    nc.tensor.transpose(
                        xnt_ps[:],
                        xn_sb[:, kc * P : (kc + 1) * P],
                        identity[:B, :B],
                    )
                    nc.scalar.mul(
                        xnt_sb[:, kc * B : (kc + 1) * B], xnt_ps[:],
                        lnw_sb[:, kc : kc + 1],
                    )

                # projections in natural form [16, 512] (lhsT = xnt chunk,
                # 16-column weight loads) -- each accumulator is alone in its
                # bank so plain start/stop group semantics are safe
                q_ps = p1ps.tile([B, NHL * HD], F32, tag="qps", bufs=1)
                k_ps = p1ps.tile([B, NHL * HD], F32, tag="kps", bufs=1)
                v_ps = p1ps.tile([B, NHL * HD], F32, tag="vps", bufs=1)
                for kc in range(NKC):
                    wq_sb = wpool.tile([P, NHL * HD], BF16, tag="wq")
                    nc.sync.dma_start(wq_sb[:], wq_d[kc * P : (kc + 1) * P, :])
                    wk_sb = wpool.tile([P, NHL * HD], BF16, tag="wk")
                    nc.sync.dma_start(wk_sb[:], wk_d[kc * P : (kc + 1) * P, :])
                    wv_sb = wpool.tile([P, NHL * HD], BF16, tag="wv")
                    nc.sync.dma_start(wv_sb[:], wv_d[kc * P : (kc + 1) * P, :])
                    xnt_c = xnt_sb[:, kc * B : (kc + 1) * B]
                    nc.tensor.matmul(
                        q_ps[:], xnt_c, wq_sb[:],
                        start=(kc == 0), stop=(kc == NKC - 1),
                    )
                    nc.tensor.matmul(
                        k_ps[:], xnt_c, wk_sb[:],
                        start=(kc == 0), stop=(kc == NKC - 1),
                    )
                    nc.tensor.matmul(
                        v_ps[:], xnt_c, wv_sb[:],
                        start=(kc == 0), stop=(kc == NKC - 1),
                    )
                q_nat = p1sb.tile([B, NHL * HD], F32, tag="qnat")
                nc.scalar.copy(q_nat[:], q_ps[:])
                k_nat = p1sb.tile([B, NHL * HD], F32, tag="knat")
                nc.scalar.copy(k_nat[:], k_ps[:])
                nc.scalar.copy(v_sb[:], v_ps[:])
                for h in range(NHL):
                    qh_ps = p1ps.tile([P, B], F32, tag="xntps", bufs=2)
                    nc.tensor.transpose(
                        qh_ps[:], q_nat[:, h * HD : (h + 1) * HD],
                        identity[:B, :B],
                    )
                    nc.scalar.copy(qt_sb[:, h * B : (h + 1) * B], qh_ps[:])
                    kh_ps = p1ps.tile([P, B], F32, tag="xntps", bufs=2)
                    nc.tensor.transpose(
                        kh_ps[:], k_nat[:, h * HD : (h + 1) * HD],
                        identity[:B, :B],
                    )
                    nc.scalar.copy(kt_sb[:, h * B : (h + 1) * B], kh_ps[:])

            # ---------------- pass K: scores ----------------
            sc_cm = tc.tile_pool(name="sc", bufs=1, space="PSUM")
            sc_pool = sc_cm.__enter__()
            zatt_cm = tc.tile_pool(name="zatt", bufs=1, space="PSUM")
            zatt_pool = zatt_cm.__enter__()
            # persistent score PSUM: 8 j-tiles of [128, 64] packed per bank
            n_sc_banks = (jmax + 7) // 8
            sc_ps = [
                sc_pool.tile([P, 512], F32, tag=f"sc{i}", name=f"sc{i}")
                for i in range(n_sc_banks)
            ]
            z_ps = zatt_pool.tile([1, B * NHL], F32, tag="z")
            attn_ps = zatt_pool.tile([P, B * NHL], F32, tag="attn")

            def sc_slice(j, c0, c1, p0, p1):
                return sc_ps[j // 8][p0:p1, (j % 8) * 64 + c0 : (j % 8) * 64 + c1]

            # memset all score banks to -1e30 (invalid rows stay masked)
            for t_ in sc_ps:
                nc.vector.memset(t_[:], -1.0e30)
            nc.tensor.matmul(
                z_ps[:], zrow[0:1, 0:1], zrow[0:1, : B * NHL],
                start=True, stop=True, skip_group_check=True,
            )
            nc.tensor.matmul(
                attn_ps[:], zrow[0:1, :P], zrow[0:1, : B * NHL],
                start=True, stop=True, skip_group_check=True,
            )

            with (
                tc.tile_pool(name="kpool", bufs=6) as kpool,
                tc.tile_pool(name="ktr", bufs=8) as ktr_pool,
                tc.tile_pool(name="ktrps", bufs=4, space="PSUM") as ktrps_pool,
            ):
                for b in range(B):
                    L = Ls[b]
                    ngrp = (nblk[b] + JG - 1) // JG
                    for jg in range(ngrp):
                        rows_g = max(0, min(JG * P, L - jg * JG * P))
                        q128, rem = divmod(rows_g, P)
                        if rows_g > 0:
                            k_tile = kpool.tile([P, JG * NHL * HD], BF16, tag="k")
                            if q128 > 0:
                                src = kc_d[
                                    b, jg * JG * P : jg * JG * P + q128 * P, :
                                ].rearrange("(jj p) d -> p jj d", p=P)
                                nc.sync.dma_start(
                                    k_tile[:, : q128 * NHL * HD].rearrange(
                                        "p (jj d) -> p jj d", d=NHL * HD
                                    ),
                                    src,
                                )
                            if rem > 0:
                                nc.sync.dma_start(
                                    k_tile[
                                        :rem,
                                        q128 * NHL * HD : (q128 + 1) * NHL * HD,
                                    ],
                                    kc_d[
                                        b,
                                        jg * JG * P + q128 * P : jg * JG * P + rows_g,
                                        :,
                                    ],
                                )
                        else:
                            k_tile = None
                        for jj in range(JG):
                            j = jg * JG + jj
                            if j >= nblk[b]:
                                break
                            tail = j == nblk[b] - 1
                            sub = min(P, max(0, L - j * P))  # cache rows here
                            m = vt[b] if tail else P        # scores to emit
                            for h in range(NHL):
                                ktr_sb = ktr_pool.tile([P, P], BF16, tag="ktr")
                                if sub > 0:
                                    ktr_ps = ktrps_pool.tile([P, P], BF16, tag="ktrp")
                                    nc.tensor.transpose(
                                        ktr_ps[:, :sub],
                                        k_tile[
                                            :sub,
                                            jj * NHL * HD + h * HD : jj * NHL * HD + (h + 1) * HD,
                                        ],
                                        identity_bf[:sub, :sub],
                                    )
                                    # alternate evacuation between ACT and DVE
                                    if (b + j + h) % 2 == 0:
                                        nc.scalar.copy(
                                            ktr_sb[:, :sub], ktr_ps[:, :sub]
                                        )
                                    else:
                                        nc.vector.tensor_copy(
                                            ktr_sb[:, :sub], ktr_ps[:, :sub]
                                        )
                                if tail:
                                    nc.vector.tensor_copy(
                                        ktr_sb[:, r[b] : r[b] + 1],
                                        kt_sb[:, h * B + b : h * B + b + 1],
                                    )
                                col = b * NHL + h
                                nc.tensor.matmul(
                                    sc_slice(j, col, col + 1, 0, m),
                                    ktr_sb[:, :m],
                                    qt_sb[:, h * B + b : h * B + b + 1],
                                    start=True, stop=True,
                                )

            # ---------------- softmax (no max-sub) ----------------
            p_cols = []
            for j in range(jmax):
                pc = p_pool.tile([P, B * NHL], BF16, tag=f"p{j}", name=f"p{j}")
                nc.scalar.activation(
                    pc[:, : NHL * nbj[j]],
                    sc_slice(j, 0, NHL * nbj[j], 0, P),
                    mybir.ActivationFunctionType.Exp,
                    scale=SCALE,
                )
                p_cols.append(pc)
            for j in range(jmax):
                nc.tensor.matmul(
                    z_ps[0:1, : NHL * nbj[j]],
                    ones_col[:],
                    p_cols[j][:, : NHL * nbj[j]],
                    start=False, stop=(j == jmax - 1),
                    skip_group_check=True,
                )
            invz_row = persist.tile([1, B * NHL], F32, tag="invzr")
            nc.vector.reciprocal(invz_row[:], z_ps[:])
            # bounce [1, 64] -> DRAM -> [16, 4] (free dim cannot become a
            # partition dim within SBUF without a transpose; DRAM is linear)
            invz_dram = nc.dram_tensor("invz_scratch", [1, B * NHL], F32)
            nc.gpsimd.dma_start(invz_dram[:, :], invz_row[:])
            invz_nat = persist.tile([B, NHL], F32, tag="invzn")
            nc.gpsimd.dma_start(
                invz_nat[:],
                invz_dram.rearrange("o (b h) -> (o b) h", b=B),
            )

            # ---------------- pass V: attn = p @ V ----------------
            with tc.tile_pool(name="vpool", bufs=6) as vpool:
                for b in range(B):
                    L = Ls[b]
                    ngrp = (nblk[b] + JG - 1) // JG
                    for jg in range(ngrp):
                        rows_g = max(0, min(JG * P, L - jg * JG * P))
                        q128, rem = divmod(rows_g, P)
                        v_tile = vpool.tile([P, JG * NHL * HD], BF16, tag="v")
                        if q128 > 0:
                            src = vc_d[
                                b, jg * JG * P : jg * JG * P + q128 * P, :
                            ].rearrange("(jj p) d -> p jj d", p=P)
                            nc.sync.dma_start(
                                v_tile[:, : q128 * NHL * HD].rearrange(
                                    "p (jj d) -> p jj d", d=NHL * HD
                                ),
                                src,
                            )
                        if rem > 0:
                            nc.sync.dma_start(
                                v_tile[
                                    :rem, q128 * NHL * HD : (q128 + 1) * NHL * HD
                                ],
                                vc_d[
                                    b,
                                    jg * JG * P + q128 * P : jg * JG * P + rows_g,
                                    :,
                                ],
                            )
                        for jj in range(JG):
                            j = jg * JG + jj
                            if j >= nblk[b]:
                                break
                            tail = j == nblk[b] - 1
                            m = vt[b] if tail else P
                            if tail:
                                # splice the new token's v row in
                                nc.gpsimd.dma_start(
                                    v_tile[
                                        r[b] : r[b] + 1,
                                        jj * NHL * HD : (jj + 1) * NHL * HD,
                                    ],
                                    v_sb[b : b + 1, :],
                                )
                            for h in range(NHL):
                                col = b * NHL + h
                                # attn columns are head-major so o_proj's lhsT
                                # per head is a contiguous [128, 16] slice
                                nc.tensor.matmul(
                                    attn_ps[:, h * B + b : h * B + b + 1],
                                    v_tile[
                                        :m,
                                        jj * NHL * HD + h * HD : jj * NHL * HD + (h + 1) * HD,
                                    ],
                                    p_cols[j][:m, col : col + 1],
                                    start=False, stop=tail,
                                    skip_group_check=True,
                                )

            attn_sb = persist.tile([P, B * NHL], BF16, tag="attnsb")
            nc.scalar.copy(attn_sb[:], attn_ps[:])

            # ---------------- o_proj partial + 1/Z ----------------
            out_sb = persist.tile([B, D], F32, tag="outsb")
            with (
                tc.tile_pool(name="wopool", bufs=4) as wopool,
                tc.tile_pool(name="ops", bufs=4, space="PSUM") as o_ps_pool,
                tc.tile_pool(name="osb", bufs=2) as o_sb_pool,
            ):
                NCH = D // 512
                for nch in range(NCH):
                    scaled = []
                    for h in range(NHL):
                        wo_sb = wopool.tile([P, 512], BF16, tag="wo")
                        nc.sync.dma_start(
                            wo_sb[:],
                            wo_d[h * HD : (h + 1) * HD, nch * 512 : (nch + 1) * 512],
                        )
                        o_ps = o_ps_pool.tile([B, 512], F32, tag="ops")
                        nc.tensor.matmul(
                            o_ps[:],
                            attn_sb[:, h * B : (h + 1) * B],
                            wo_sb[:],
                            start=True, stop=True,
                        )
                        s = o_sb_pool.tile([B, 512], F32, tag=f"os{h}")
                        nc.scalar.mul(s[:], o_ps[:], invz_nat[:, h : h + 1])
                        scaled.append(s)
                    acc = out_sb[:, nch * 512 : (nch + 1) * 512]
                    nc.vector.tensor_add(acc, scaled[0][:], scaled[1][:])
                    nc.vector.tensor_add(acc, acc, scaled[2][:])
                    nc.vector.tensor_add(acc, acc, scaled[3][:])
            nc.sync.dma_start(out_d[:, :], out_sb[:])
            zatt_cm.__exit__(None, None, None)
            sc_cm.__exit__(None, None, None)

    _split_excess_waits(nc)
    return nc


def _prep_inputs(x, ln_w, Wq, Wk, Wv, Wo, K_cache, V_cache, cache_lens):
    x = np.asarray(x, np.float32).reshape(B, D)
    ln_w = np.asarray(ln_w, np.float32)
    cache_lens = np.asarray(cache_lens, np.int32)
    perm = np.argsort(-cache_lens, kind="stable")
    Ls = [int(cache_lens[p]) for p in perm]
    lnw2d = np.ascontiguousarray(ln_w.reshape(NKC, P).T)
    x_s = np.ascontiguousarray(x[perm])
    K4 = np.asarray(K_cache, np.float32).reshape(B, T, H, HD)
    V4 = np.asarray(V_cache, np.float32).reshape(B, T, H, HD)
    in_maps = []
    for c in range(NCORES):
        h0 = c * NHL
        in_maps.append(
            {
                "x": x_s,
                "lnw": lnw2d,
                "wq": np.ascontiguousarray(
                    np.asarray(Wq, np.float32)[:, h0 * HD : (h0 + NHL) * HD]
                ).astype(BF16_NP),
                "wk": np.ascontiguousarray(
                    np.asarray(Wk, np.float32)[:, h0 * HD : (h0 + NHL) * HD]
                ).astype(BF16_NP),
                "wv": np.ascontiguousarray(
                    np.asarray(Wv, np.float32)[:, h0 * HD : (h0 + NHL) * HD]
                ).astype(BF16_NP),
                "wo": np.ascontiguousarray(
                    np.asarray(Wo, np.float32)[h0 * HD : (h0 + NHL) * HD, :]
                ).astype(BF16_NP),
                "kc": np.ascontiguousarray(
                    K4[perm][:, :, h0 : h0 + NHL, :]
                ).reshape(B, T, NHL * HD).astype(BF16_NP),
                "vc": np.ascontiguousarray(
                    V4[perm][:, :, h0 : h0 + NHL, :]
                ).reshape(B, T, NHL * HD).astype(BF16_NP),
            }
        )
    return in_maps, Ls, perm, x_s


def _run(x, ln_w, Wq, Wk, Wv, Wo, K_cache, V_cache, cache_lens, trace=False):
    in_maps, Ls, perm, x_s = _prep_inputs(
        x, ln_w, Wq, Wk, Wv, Wo, K_cache, V_cache, cache_lens
    )
    nc = _build(Ls)
    res = run_bass_kernel_spmd(
        nc, in_maps, core_ids=list(range(NCORES)), trace=trace
    )
    partial = np.zeros((B, D), np.float32)
    for c in range(NCORES):
        partial += res.results[c]["out"]
    out_sorted = x_s + partial
    out = np.empty((B, D), np.float32)
    out[perm] = out_sorted
    return out.reshape(B, 1, D), res


def kernel(x, ln_w, Wq, Wk, Wv, Wo, K_cache, V_cache, cache_lens):
    out, _ = _run(x, ln_w, Wq, Wk, Wv, Wo, K_cache, V_cache, cache_lens)
    return out
